# revision 8
# baseline (speedup 1.0000x reference)
"""Causal self-attention (dense transformer block) on 8 TRN2 NeuronCores.

Tensor-parallel over heads: 16 heads / 8 cores -> 2 heads per core, both
batch elements on every core. Per core:
  - QKV projection in "T layout": q^T/k^T per head [dh, tok] (f32r, token
    chunks of 512 so matmul time hides LDWEIGHTS), V natural [tok, dh] in
    bf16; q/k biases fused into the PSUM eviction on ACT
  - causal attention with scores in transposed layout [k, q], chunk-outer
    loop (c outer, kb inner) so PSUM accumulation chains are short:
      * scores via PE (f32r), diagonal blocks column-trimmed (ap = 512-off)
      * tril mask add on DVE (diagonal blocks only)
      * exp on ACT (scale folded), output bf16
      * denominator: ones^T @ p accumulated into a [1, 512] PSUM row by the
        PE alongside the attn@V chain (no DVE rowsum adds)
      * attn@V (bf16 inputs) accumulates in the [dh, q] layout out_proj needs
      * postlude per chunk: broadcast denominators across partitions on the
        otherwise-idle GpSimd engine, reciprocal on DVE, then a fused
        normalize+evict of the attn output via scalar_tensor_tensor
  - out_proj: both heads accumulate into ONE PSUM bank (u pre-normalized),
    evictions cast to bf16 split between ACT and DVE, one batched DMA per
    token block
  - all biases (b_out and the V-bias term) are folded on the host into the
    final gather; core partials are written as bf16 to halve the DMA tail
Matmuls run as float32r / bf16 (full PE rate at free dim >= 256).
"""
import sys

if "/opt/trn_rl_repo" not in sys.path:
    sys.path.insert(0, "/opt/trn_rl_repo")

import numpy as np

import concourse.bacc as bacc
import concourse.bass as bass
import concourse.mybir as mybir
import concourse.tile as tile
from concourse.bass_utils import run_bass_kernel_spmd

P = 128
B, S, D = 2, 2048, 2048
H, DH = 16, 128
HPC = 2            # heads per core
NCORES = 8
TC = 512           # token chunk for the QKV projection
QC = 512           # q chunk for attention
NQC = S // QC      # 4 q chunks
KPQ = QC // P      # 4 key blocks per q chunk
SCALE = 1.0 / float(np.sqrt(DH))

f32 = mybir.dt.float32
f32r = mybir.dt.float32r
bf16 = mybir.dt.bfloat16
Act = mybir.ActivationFunctionType
Alu = mybir.AluOpType


def _emit(nc, tc_ctx, aps):
    xty, wqkv, bqk, wout, trilm, onesc, out_p = aps
    tc = tc_ctx
    NTB = S // P            # 16 token blocks per batch
    NDC = D // P            # 16 contraction chunks

    with (
        tc.tile_pool(name="const", bufs=1) as const,
        tc.tile_pool(name="xtp", bufs=2) as xtp,
        tc.tile_pool(name="qk", bufs=1) as qk,
        tc.tile_pool(name="vp", bufs=1) as vp,
        tc.tile_pool(name="pp", bufs=4) as pp,
        tc.tile_pool(name="up", bufs=1) as up,
        tc.tile_pool(name="sdp", bufs=2) as sdp,
        tc.tile_pool(name="rbp", bufs=1) as rbp,
        tc.tile_pool(name="fin", bufs=2) as fin,
        tc.tile_pool(name="ps_a", bufs=3, space="PSUM") as ps_a,
        tc.tile_pool(name="ps_u", bufs=3, space="PSUM") as ps_u,
        tc.tile_pool(name="ps_d", bufs=2, space="PSUM") as ps_d,
    ):
        bqk_sb = const.tile([P, 4], f32)
        nc.sync.dma_start(bqk_sb, bqk)
        w_sb = const.tile([P, NDC, 6 * P], f32r)
        wqkv_r = wqkv.rearrange("(dc p) c -> p dc c", p=P).bitcast(f32r)
        for dc in range(NDC):
            nc.sync.dma_start(w_sb[:, dc, :], wqkv_r[:, dc, :])
        tril_sb = const.tile([P, P], f32)
        onec_sb = const.tile([P, 1], bf16)
        wo_sb = const.tile([P, HPC, D], f32r)

        def load_late_consts():
            nc.sync.dma_start(tril_sb, trilm)
            nc.sync.dma_start(onec_sb, onesc)
            nc.sync.dma_start(
                wo_sb, wout.rearrange("(h p) c -> p h c", p=P).bitcast(f32r)
            )

        for b in range(B):
            # ---------------- QKV projection ----------------
            q_sb = [qk.tile([P, S], f32r, tag=f"q{h}", name=f"q{h}") for h in range(HPC)]
            k_sb = [qk.tile([P, S], f32r, tag=f"k{h}", name=f"k{h}") for h in range(HPC)]
            v_sb = vp.tile([P, NTB, HPC * DH], bf16, tag="v", name="v_sb")

            for tci in range(S // TC):
                xt = xtp.tile([P, NDC, TC], f32r, tag="xt", name="xt")
                xsrc = (
                    xty[b, :, tci * TC:(tci + 1) * TC]
                    .rearrange("(dc p) t -> p dc t", p=P)
                    .bitcast(f32r)
                )
                if b == 0 and tci == 0:
                    # split the first chunk so early dc matmuls start sooner
                    for g in range(4):
                        nc.sync.dma_start(
                            xt[:, g * 4:(g + 1) * 4, :], xsrc[:, g * 4:(g + 1) * 4, :]
                        )
                else:
                    nc.sync.dma_start(xt, xsrc)
                # q^T / k^T for both heads: psum [col=128, tok=TC]
                for cb in range(4):
                    psq = ps_a.tile([P, TC], f32, tag="a", name="psq")
                    for dc in range(NDC):
                        nc.tensor.matmul(
                            psq,
                            w_sb[:, dc, cb * P:(cb + 1) * P],
                            xt[:, dc, :],
                            start=(dc == 0),
                            stop=(dc == NDC - 1),
                        )
                    dst = q_sb[cb] if cb < HPC else k_sb[cb - HPC]
                    nc.scalar.activation(
                        dst[:, tci * TC:(tci + 1) * TC],
                        psq,
                        Act.Identity,
                        bias=bqk_sb[:, cb:cb + 1],
                    )
                # V natural for both heads: psum [tok=128, 2*dh]
                for tb in range(TC // P):
                    psv = ps_a.tile([P, HPC * DH], f32, tag="a", name="psv")
                    for dc in range(NDC):
                        nc.tensor.matmul(
                            psv,
                            xt[:, dc, tb * P:(tb + 1) * P],
                            w_sb[:, dc, 4 * P:6 * P],
                            start=(dc == 0),
                            stop=(dc == NDC - 1),
                        )
                    nc.scalar.copy(v_sb[:, tci * (TC // P) + tb, :], psv)

            if b == 0:
                load_late_consts()

            # ---------------- attention per head ----------------
            u_sb = []
            for h in range(HPC):
                us = up.tile([P, S], f32r, tag=f"u{h}", name=f"u{h}")
                for c in range(NQC):
                    psu = ps_u.tile([P, QC], f32, tag="u", name="psu")
                    psd = ps_d.tile([1, QC], f32, tag="d", name="psd")
                    last_kb = KPQ * c + KPQ - 1
                    for kb in range(last_kb + 1):
                        diag = kb >= KPQ * c
                        off = (kb - KPQ * c) * P if diag else 0
                        psp = ps_a.tile([P, QC], f32, tag="a", name="psp")
                        nc.tensor.matmul(
                            psp[:, off:],
                            k_sb[h][:, kb * P:(kb + 1) * P],
                            q_sb[h][:, c * QC + off:(c + 1) * QC],
                            start=True,
                            stop=True,
                        )
                        if diag:
                            nc.vector.tensor_add(
                                psp[:, off:off + P],
                                psp[:, off:off + P],
                                tril_sb,
                            )
                        p_t = pp.tile([P, QC], bf16, tag="p", name="p_t")
                        nc.scalar.activation(
                            p_t[:, off:], psp[:, off:], Act.Exp, scale=SCALE
                        )
                        # denominator: ones^T @ p -> [1, qc], accumulated
                        nc.tensor.matmul(
                            psd[:, off:],
                            onec_sb,
                            p_t[:, off:],
                            start=(kb == 0),
                            stop=(kb == last_kb),
                        )
                        # attn @ V in [dh, q] layout, accumulated
                        nc.tensor.matmul(
                            psu[:, off:],
                            v_sb[:, kb, h * DH:(h + 1) * DH],
                            p_t[:, off:],
                            start=(kb == 0),
                            stop=(kb == last_kb),
                        )
                    # ---- chunk postlude: broadcast 1/denominator, evict u
                    sdr = sdp.tile([1, QC], f32, tag="s", name="sdr")
                    nc.scalar.copy(sdr, psd)
                    rbb = rbp.tile([P, QC], f32, tag="rbb", name="rbb")
                    nc.gpsimd.partition_broadcast(rbb, sdr)
                    rb_sb = rbp.tile([P, QC], f32, tag="rb", name="rb_sb")
                    nc.vector.reciprocal_approx_fast(out=rb_sb, in_=rbb)
                    nc.vector.scalar_tensor_tensor(
                        us[:, c * QC:(c + 1) * QC],
                        psu,
                        1.0,
                        rb_sb,
                        op0=Alu.mult,
                        op1=Alu.mult,
                    )
                u_sb.append(us)

            # ---------------- out projection ----------------
            u0 = u_sb[0]
            u1 = u_sb[1]
            for tb in range(NTB):
                f_t = fin.tile([P, D], bf16, tag="f", name="f_t")
                for cc in range(D // QC):
                    pso = ps_a.tile([P, QC], f32, tag="a", name="pso")
                    nc.tensor.matmul(
                        pso,
                        u0[:, tb * P:(tb + 1) * P],
                        wo_sb[:, 0, cc * QC:(cc + 1) * QC],
                        start=True,
                        stop=False,
                    )
                    nc.tensor.matmul(
                        pso,
                        u1[:, tb * P:(tb + 1) * P],
                        wo_sb[:, 1, cc * QC:(cc + 1) * QC],
                        start=False,
                        stop=True,
                    )
                    dst = f_t[:, cc * QC:(cc + 1) * QC]
                    if cc % 2 == 0:
                        nc.scalar.copy(dst, pso)
                    else:
                        nc.vector.tensor_copy(out=dst, in_=pso)
                nc.sync.dma_start(out_p[b, tb * P:(tb + 1) * P, :], f_t)


_CACHE = {}


def _build():
    if "nc" in _CACHE:
        return _CACHE["nc"]
    nc = bacc.Bacc("TRN2", target_bir_lowering=False, debug=False)
    xty = nc.dram_tensor("xty", [B, D, S], f32, kind="ExternalInput").ap()
    wqkv = nc.dram_tensor("wqkv", [D, 6 * P], f32, kind="ExternalInput").ap()
    bqk = nc.dram_tensor("bqk", [P, 4], f32, kind="ExternalInput").ap()
    wout = nc.dram_tensor("wout", [HPC * DH, D], f32, kind="ExternalInput").ap()
    trilm = nc.dram_tensor("trilm", [P, P], f32, kind="ExternalInput").ap()
    onesc = nc.dram_tensor("onesc", [P, 1], bf16, kind="ExternalInput").ap()
    out_p = nc.dram_tensor("out_p", [B, S, D], bf16, kind="ExternalOutput").ap()
    with tile.TileContext(nc) as tctx:
        _emit(nc, tctx, (xty, wqkv, bqk, wout, trilm, onesc, out_p))
    nc.compile()
    _CACHE["nc"] = nc
    return nc


def _in_maps(x, W_qkv, b_qkv, W_out, b_out):
    import ml_dtypes
    trilm = np.where(
        np.arange(P)[None, :] >= np.arange(P)[:, None], 0.0, -1e9
    ).astype(np.float32)
    onesc = np.ones((P, 1), dtype=ml_dtypes.bfloat16)
    xty = np.ascontiguousarray(x.transpose(0, 2, 1))
    maps = []
    for core in range(NCORES):
        h0 = core * HPC
        cols = []
        for off in (0, D, 2 * D):  # q, k, v column groups of W_qkv
            for h in range(h0, h0 + HPC):
                cols.append((off + h * DH, off + (h + 1) * DH))
        wqkv_c = np.concatenate(
            [W_qkv[:, a:b_] for a, b_ in cols], axis=1
        ).astype(np.float32)
        bqk_c = np.stack(
            [b_qkv[a:b_] for a, b_ in cols[:4]], axis=1
        ).astype(np.float32)  # [128, 4]
        wout_c = W_out[h0 * DH:(h0 + HPC) * DH, :].astype(np.float32)
        maps.append({
            "xty": xty,
            "wqkv": np.ascontiguousarray(wqkv_c),
            "bqk": np.ascontiguousarray(bqk_c),
            "wout": np.ascontiguousarray(wout_c),
            "trilm": trilm,
            "onesc": onesc,
        })
    return maps


def kernel(x, W_qkv, b_qkv, W_out, b_out, _trace=False, _trace_kwargs=None):
    x = np.asarray(x, dtype=np.float32)
    W_qkv = np.asarray(W_qkv, dtype=np.float32)
    b_qkv = np.asarray(b_qkv, dtype=np.float32)
    W_out = np.asarray(W_out, dtype=np.float32)
    b_out = np.asarray(b_out, dtype=np.float32)

    nc = _build()
    maps = _in_maps(x, W_qkv, b_qkv, W_out, b_out)
    res = run_bass_kernel_spmd(
        nc, maps, core_ids=list(range(NCORES)), trace=_trace,
        **(_trace_kwargs or {}),
    )
    out = res.results[0]["out_p"].astype(np.float32)
    for c in range(1, NCORES):
        out = out + res.results[c]["out_p"].astype(np.float32)
    # all biases folded on the host: b_out + (v-bias @ W_out); exact because
    # softmax rows sum to 1, so the v-bias passes through attention unchanged
    bias_total = (b_out + b_qkv[2 * D:] @ W_out).astype(np.float32)
    out = out + bias_total[None, None, :]
    if _trace:
        _CACHE["last_results"] = res
    return out.astype(np.float32)


# revision 9
# speedup vs baseline: 1.2178x; 1.2178x over previous
"""Causal self-attention (dense transformer block) on 8 TRN2 NeuronCores.

Tensor-parallel over heads: 16 heads / 8 cores -> 2 heads per core, both
batch elements on every core. Per core:
  - QKV projection in "T layout": q^T/k^T per head [dh, tok] (f32r, token
    chunks of 512 so matmul time hides LDWEIGHTS), V natural [tok, dh] in
    bf16; q/k biases fused into the PSUM eviction on ACT
  - causal attention with scores in transposed layout [k, q], chunk-outer
    loop (c outer, kb inner) so PSUM accumulation chains are short:
      * scores via PE (f32r), diagonal blocks column-trimmed (ap = 512-off)
      * tril mask add on DVE (diagonal blocks only)
      * exp on ACT (scale folded), output bf16
      * denominator: ones^T @ p accumulated into a [1, 512] PSUM row by the
        PE alongside the attn@V chain (no DVE rowsum adds)
      * attn@V (bf16 inputs) accumulates in the [dh, q] layout out_proj needs
      * postlude per chunk: broadcast denominators across partitions on the
        otherwise-idle GpSimd engine, reciprocal on DVE, then a fused
        normalize+evict of the attn output via scalar_tensor_tensor
  - out_proj: both heads accumulate into ONE PSUM bank (u pre-normalized),
    evictions cast to bf16 split between ACT and DVE, one batched DMA per
    token block
  - all biases (b_out and the V-bias term) are folded on the host into the
    final gather; core partials are written as bf16 to halve the DMA tail
Matmuls run as float32r / bf16 (full PE rate at free dim >= 256).
"""
import sys

if "/opt/trn_rl_repo" not in sys.path:
    sys.path.insert(0, "/opt/trn_rl_repo")

import numpy as np

import concourse.bacc as bacc
import concourse.bass as bass
import concourse.mybir as mybir
import concourse.tile as tile
from concourse.bass_utils import run_bass_kernel_spmd

P = 128
B, S, D = 2, 2048, 2048
H, DH = 16, 128
HPC = 2            # heads per core
NCORES = 8
TC = 512           # token chunk for the QKV projection
QC = 512           # q chunk for attention
NQC = S // QC      # 4 q chunks
KPQ = QC // P      # 4 key blocks per q chunk
SCALE = 1.0 / float(np.sqrt(DH))

f32 = mybir.dt.float32
f32r = mybir.dt.float32r
bf16 = mybir.dt.bfloat16
Act = mybir.ActivationFunctionType
Alu = mybir.AluOpType


def _emit(nc, tc_ctx, aps):
    xty, wqkv, bqk, wout, trilm, onesc, out_p = aps
    tc = tc_ctx
    NTB = S // P            # 16 token blocks per batch
    NDC = D // P            # 16 contraction chunks

    with (
        tc.tile_pool(name="const", bufs=1) as const,
        tc.tile_pool(name="xtp", bufs=2) as xtp,
        tc.tile_pool(name="qk", bufs=1) as qk,
        tc.tile_pool(name="vp", bufs=1) as vp,
        tc.tile_pool(name="pp", bufs=4) as pp,
        tc.tile_pool(name="up", bufs=1) as up,
        tc.tile_pool(name="sdp", bufs=2) as sdp,
        tc.tile_pool(name="rbp", bufs=1) as rbp,
        tc.tile_pool(name="fin", bufs=2) as fin,
        tc.tile_pool(name="ps_a", bufs=4, space="PSUM") as ps_a,
        tc.tile_pool(name="ps_u", bufs=2, space="PSUM") as ps_u,
        tc.tile_pool(name="ps_d", bufs=2, space="PSUM") as ps_d,
    ):
        bqk_sb = const.tile([P, 4], f32)
        nc.sync.dma_start(bqk_sb, bqk)
        w_sb = const.tile([P, NDC, 6 * P], f32r)
        wqkv_r = wqkv.rearrange("(dc p) c -> p dc c", p=P).bitcast(f32r)
        for dc in range(NDC):
            nc.sync.dma_start(w_sb[:, dc, :], wqkv_r[:, dc, :])
        tril_sb = const.tile([P, P], f32)
        onec_sb = const.tile([P, 1], bf16)
        wo_sb = const.tile([P, HPC, D], f32r)

        def load_late_consts():
            nc.sync.dma_start(tril_sb, trilm)
            nc.sync.dma_start(onec_sb, onesc)
            nc.sync.dma_start(
                wo_sb, wout.rearrange("(h p) c -> p h c", p=P).bitcast(f32r)
            )

        for b in range(B):
            # ---------------- QKV projection ----------------
            q_sb = [qk.tile([P, S], f32r, tag=f"q{h}", name=f"q{h}") for h in range(HPC)]
            k_sb = [qk.tile([P, S], f32r, tag=f"k{h}", name=f"k{h}") for h in range(HPC)]
            v_sb = vp.tile([P, NTB, HPC * DH], bf16, tag="v", name="v_sb")

            for tci in range(S // TC):
                xt = xtp.tile([P, NDC, TC], f32r, tag="xt", name="xt")
                xsrc = (
                    xty[b, :, tci * TC:(tci + 1) * TC]
                    .rearrange("(dc p) t -> p dc t", p=P)
                    .bitcast(f32r)
                )
                if b == 0 and tci == 0:
                    # split the first chunk so early dc matmuls start sooner
                    for g in range(4):
                        nc.sync.dma_start(
                            xt[:, g * 4:(g + 1) * 4, :], xsrc[:, g * 4:(g + 1) * 4, :]
                        )
                else:
                    nc.sync.dma_start(xt, xsrc)
                # q^T / k^T for both heads: psum [col=128, tok=TC]
                for cb in range(4):
                    psq = ps_a.tile([P, TC], f32, tag="a", name="psq")
                    for dc in range(NDC):
                        nc.tensor.matmul(
                            psq,
                            w_sb[:, dc, cb * P:(cb + 1) * P],
                            xt[:, dc, :],
                            start=(dc == 0),
                            stop=(dc == NDC - 1),
                        )
                    dst = q_sb[cb] if cb < HPC else k_sb[cb - HPC]
                    nc.scalar.activation(
                        dst[:, tci * TC:(tci + 1) * TC],
                        psq,
                        Act.Identity,
                        bias=bqk_sb[:, cb:cb + 1],
                    )
                # V natural for both heads: psum [tok=128, 2*dh]
                for tb in range(TC // P):
                    psv = ps_a.tile([P, HPC * DH], f32, tag="a", name="psv")
                    for dc in range(NDC):
                        nc.tensor.matmul(
                            psv,
                            xt[:, dc, tb * P:(tb + 1) * P],
                            w_sb[:, dc, 4 * P:6 * P],
                            start=(dc == 0),
                            stop=(dc == NDC - 1),
                        )
                    nc.scalar.copy(v_sb[:, tci * (TC // P) + tb, :], psv)

            if b == 0:
                load_late_consts()

            # ---------------- attention per head ----------------
            u_sb = []
            for h in range(HPC):
                us = up.tile([P, S], f32r, tag=f"u{h}", name=f"u{h}")
                for c in range(NQC):
                    psu = ps_u.tile([P, QC], f32, tag="u", name="psu")
                    psd = ps_d.tile([1, QC], f32, tag="d", name="psd")
                    last_kb = KPQ * c + KPQ - 1
                    for kb in range(last_kb + 1):
                        diag = kb >= KPQ * c
                        off = (kb - KPQ * c) * P if diag else 0
                        psp = ps_a.tile([P, QC], f32, tag="a", name="psp")
                        nc.tensor.matmul(
                            psp[:, off:],
                            k_sb[h][:, kb * P:(kb + 1) * P],
                            q_sb[h][:, c * QC + off:(c + 1) * QC],
                            start=True,
                            stop=True,
                        )
                        if diag:
                            nc.vector.tensor_add(
                                psp[:, off:off + P],
                                psp[:, off:off + P],
                                tril_sb,
                            )
                        p_t = pp.tile([P, QC], bf16, tag="p", name="p_t")
                        nc.scalar.activation(
                            p_t[:, off:], psp[:, off:], Act.Exp, scale=SCALE
                        )
                        # denominator: ones^T @ p -> [1, qc], accumulated
                        nc.tensor.matmul(
                            psd[:, off:],
                            onec_sb,
                            p_t[:, off:],
                            start=(kb == 0),
                            stop=(kb == last_kb),
                        )
                        # attn @ V in [dh, q] layout, accumulated
                        nc.tensor.matmul(
                            psu[:, off:],
                            v_sb[:, kb, h * DH:(h + 1) * DH],
                            p_t[:, off:],
                            start=(kb == 0),
                            stop=(kb == last_kb),
                        )
                    # ---- chunk postlude: broadcast 1/denominator, evict u
                    sdr = sdp.tile([1, QC], f32, tag="s", name="sdr")
                    nc.scalar.copy(sdr, psd)
                    rbb = rbp.tile([P, QC], f32, tag="rbb", name="rbb")
                    nc.gpsimd.partition_broadcast(rbb, sdr)
                    rb_sb = rbp.tile([P, QC], f32, tag="rb", name="rb_sb")
                    nc.vector.reciprocal_approx_fast(out=rb_sb, in_=rbb)
                    nc.vector.scalar_tensor_tensor(
                        us[:, c * QC:(c + 1) * QC],
                        psu,
                        1.0,
                        rb_sb,
                        op0=Alu.mult,
                        op1=Alu.mult,
                    )
                u_sb.append(us)

            # ---------------- out projection ----------------
            u0 = u_sb[0]
            u1 = u_sb[1]
            for tb in range(NTB):
                f_t = fin.tile([P, D], bf16, tag="f", name="f_t")
                for cc in range(D // QC):
                    pso = ps_a.tile([P, QC], f32, tag="a", name="pso")
                    nc.tensor.matmul(
                        pso,
                        u0[:, tb * P:(tb + 1) * P],
                        wo_sb[:, 0, cc * QC:(cc + 1) * QC],
                        start=True,
                        stop=False,
                    )
                    nc.tensor.matmul(
                        pso,
                        u1[:, tb * P:(tb + 1) * P],
                        wo_sb[:, 1, cc * QC:(cc + 1) * QC],
                        start=False,
                        stop=True,
                    )
                    dst = f_t[:, cc * QC:(cc + 1) * QC]
                    if cc % 2 == 0:
                        nc.scalar.copy(dst, pso)
                    else:
                        nc.vector.tensor_copy(out=dst, in_=pso)
                nc.sync.dma_start(out_p[b, tb * P:(tb + 1) * P, :], f_t)


_CACHE = {}


def _build():
    if "nc" in _CACHE:
        return _CACHE["nc"]
    nc = bacc.Bacc("TRN2", target_bir_lowering=False, debug=False)
    xty = nc.dram_tensor("xty", [B, D, S], f32, kind="ExternalInput").ap()
    wqkv = nc.dram_tensor("wqkv", [D, 6 * P], f32, kind="ExternalInput").ap()
    bqk = nc.dram_tensor("bqk", [P, 4], f32, kind="ExternalInput").ap()
    wout = nc.dram_tensor("wout", [HPC * DH, D], f32, kind="ExternalInput").ap()
    trilm = nc.dram_tensor("trilm", [P, P], f32, kind="ExternalInput").ap()
    onesc = nc.dram_tensor("onesc", [P, 1], bf16, kind="ExternalInput").ap()
    out_p = nc.dram_tensor("out_p", [B, S, D], bf16, kind="ExternalOutput").ap()
    with tile.TileContext(nc) as tctx:
        _emit(nc, tctx, (xty, wqkv, bqk, wout, trilm, onesc, out_p))
    nc.compile()
    _CACHE["nc"] = nc
    return nc


def _in_maps(x, W_qkv, b_qkv, W_out, b_out):
    import ml_dtypes
    trilm = np.where(
        np.arange(P)[None, :] >= np.arange(P)[:, None], 0.0, -1e9
    ).astype(np.float32)
    onesc = np.ones((P, 1), dtype=ml_dtypes.bfloat16)
    xty = np.ascontiguousarray(x.transpose(0, 2, 1))
    maps = []
    for core in range(NCORES):
        h0 = core * HPC
        cols = []
        for off in (0, D, 2 * D):  # q, k, v column groups of W_qkv
            for h in range(h0, h0 + HPC):
                cols.append((off + h * DH, off + (h + 1) * DH))
        wqkv_c = np.concatenate(
            [W_qkv[:, a:b_] for a, b_ in cols], axis=1
        ).astype(np.float32)
        bqk_c = np.stack(
            [b_qkv[a:b_] for a, b_ in cols[:4]], axis=1
        ).astype(np.float32)  # [128, 4]
        wout_c = W_out[h0 * DH:(h0 + HPC) * DH, :].astype(np.float32)
        maps.append({
            "xty": xty,
            "wqkv": np.ascontiguousarray(wqkv_c),
            "bqk": np.ascontiguousarray(bqk_c),
            "wout": np.ascontiguousarray(wout_c),
            "trilm": trilm,
            "onesc": onesc,
        })
    return maps


def kernel(x, W_qkv, b_qkv, W_out, b_out, _trace=False, _trace_kwargs=None):
    x = np.asarray(x, dtype=np.float32)
    W_qkv = np.asarray(W_qkv, dtype=np.float32)
    b_qkv = np.asarray(b_qkv, dtype=np.float32)
    W_out = np.asarray(W_out, dtype=np.float32)
    b_out = np.asarray(b_out, dtype=np.float32)

    nc = _build()
    maps = _in_maps(x, W_qkv, b_qkv, W_out, b_out)
    res = run_bass_kernel_spmd(
        nc, maps, core_ids=list(range(NCORES)), trace=_trace,
        **(_trace_kwargs or {}),
    )
    out = res.results[0]["out_p"].astype(np.float32)
    for c in range(1, NCORES):
        out = out + res.results[c]["out_p"].astype(np.float32)
    # all biases folded on the host: b_out + (v-bias @ W_out); exact because
    # softmax rows sum to 1, so the v-bias passes through attention unchanged
    bias_total = (b_out + b_qkv[2 * D:] @ W_out).astype(np.float32)
    out = out + bias_total[None, None, :]
    if _trace:
        _CACHE["last_results"] = res
    return out.astype(np.float32)


# revision 10
# speedup vs baseline: 1.3126x; 1.0778x over previous
"""Causal self-attention (dense transformer block) on 8 TRN2 NeuronCores.

Tensor-parallel over heads: 16 heads / 8 cores -> 2 heads per core, both
batch elements on every core. Per core:
  - QKV projection in "T layout": q^T/k^T per head [dh, tok] (f32r, token
    chunks of 512 so matmul time hides LDWEIGHTS), V natural [tok, dh] in
    bf16; q/k biases fused into the PSUM eviction on ACT
  - causal attention with scores in transposed layout [k, q], chunk-outer
    loop (c outer, kb inner) so PSUM accumulation chains are short:
      * scores via PE (f32r), diagonal blocks column-trimmed (ap = 512-off)
      * tril mask add on DVE (diagonal blocks only)
      * exp on ACT (scale folded), output bf16
      * denominator: ones^T @ p accumulated into a [1, 512] PSUM row by the
        PE alongside the attn@V chain (no DVE rowsum adds)
      * attn@V (bf16 inputs) accumulates in the [dh, q] layout out_proj needs
      * postlude per chunk: broadcast denominators across partitions on the
        otherwise-idle GpSimd engine, reciprocal on DVE, then a fused
        normalize+evict of the attn output via scalar_tensor_tensor
  - out_proj: both heads accumulate into ONE PSUM bank (u pre-normalized),
    evictions cast to bf16 split between ACT and DVE, one batched DMA per
    token block
  - all biases (b_out and the V-bias term) are folded on the host into the
    final gather; core partials are written as bf16 to halve the DMA tail
Matmuls run as float32r / bf16 (full PE rate at free dim >= 256).
"""
import sys

if "/opt/trn_rl_repo" not in sys.path:
    sys.path.insert(0, "/opt/trn_rl_repo")

import numpy as np

import concourse.bacc as bacc
import concourse.bass as bass
import concourse.mybir as mybir
import concourse.tile as tile
from concourse.bass_utils import run_bass_kernel_spmd

P = 128
B, S, D = 2, 2048, 2048
H, DH = 16, 128
HPC = 2            # heads per core
NCORES = 8
TC = 512           # token chunk for the QKV projection
QC = 512           # q chunk for attention
NQC = S // QC      # 4 q chunks
KPQ = QC // P      # 4 key blocks per q chunk
SCALE = 1.0 / float(np.sqrt(DH))

f32 = mybir.dt.float32
f32r = mybir.dt.float32r
bf16 = mybir.dt.bfloat16
Act = mybir.ActivationFunctionType
Alu = mybir.AluOpType


def _emit(nc, tc_ctx, aps):
    xty, wqkv, bqk, wout, trilm, onesc, out_p = aps
    tc = tc_ctx
    NTB = S // P            # 16 token blocks per batch
    NDC = D // P            # 16 contraction chunks

    with (
        tc.tile_pool(name="const", bufs=1) as const,
        tc.tile_pool(name="xtp", bufs=2) as xtp,
        tc.tile_pool(name="qk", bufs=1) as qk,
        tc.tile_pool(name="vp", bufs=1) as vp,
        tc.tile_pool(name="pp", bufs=6) as pp,
        tc.tile_pool(name="up", bufs=1) as up,
        tc.tile_pool(name="rbp", bufs=2) as rbp,
        tc.tile_pool(name="fin", bufs=2) as fin,
        tc.tile_pool(name="ps_a", bufs=4, space="PSUM") as ps_a,
        tc.tile_pool(name="ps_u", bufs=2, space="PSUM") as ps_u,
        tc.tile_pool(name="ps_d", bufs=2, space="PSUM") as ps_d,
    ):
        bqk_sb = const.tile([P, 4], f32)
        nc.sync.dma_start(bqk_sb, bqk)
        w_sb = const.tile([P, NDC, 6 * P], f32r)
        wqkv_r = wqkv.rearrange("(dc p) c -> p dc c", p=P).bitcast(f32r)
        for dc in range(NDC):
            nc.sync.dma_start(w_sb[:, dc, :], wqkv_r[:, dc, :])
        tril_sb = const.tile([P, P], bf16)
        onec_sb = const.tile([P, P], bf16)
        wo_sb = const.tile([P, HPC, D], f32r)

        def load_late_consts():
            nc.sync.dma_start(tril_sb, trilm)
            nc.sync.dma_start(onec_sb, onesc)
            nc.sync.dma_start(
                wo_sb, wout.rearrange("(h p) c -> p h c", p=P).bitcast(f32r)
            )

        for b in range(B):
            # ---------------- QKV projection ----------------
            q_sb = [qk.tile([P, S], f32r, tag=f"q{h}", name=f"q{h}") for h in range(HPC)]
            k_sb = [qk.tile([P, S], f32r, tag=f"k{h}", name=f"k{h}") for h in range(HPC)]
            v_sb = vp.tile([P, NTB, HPC * DH], bf16, tag="v", name="v_sb")

            for tci in range(S // TC):
                xt = xtp.tile([P, NDC, TC], f32r, tag="xt", name="xt")
                xsrc = (
                    xty[b, :, tci * TC:(tci + 1) * TC]
                    .rearrange("(dc p) t -> p dc t", p=P)
                    .bitcast(f32r)
                )
                if b == 0 and tci == 0:
                    # split the first chunk so early dc matmuls start sooner
                    for g in range(4):
                        nc.sync.dma_start(
                            xt[:, g * 4:(g + 1) * 4, :], xsrc[:, g * 4:(g + 1) * 4, :]
                        )
                else:
                    nc.sync.dma_start(xt, xsrc)
                # q^T / k^T for both heads: psum [col=128, tok=TC]
                for cb in range(4):
                    psq = ps_a.tile([P, TC], f32, tag="a", name="psq")
                    for dc in range(NDC):
                        nc.tensor.matmul(
                            psq,
                            w_sb[:, dc, cb * P:(cb + 1) * P],
                            xt[:, dc, :],
                            start=(dc == 0),
                            stop=(dc == NDC - 1),
                        )
                    dst = q_sb[cb] if cb < HPC else k_sb[cb - HPC]
                    nc.scalar.activation(
                        dst[:, tci * TC:(tci + 1) * TC],
                        psq,
                        Act.Identity,
                        bias=bqk_sb[:, cb:cb + 1],
                    )
                # V natural for both heads: psum [tok=128, 2*dh]
                for tb in range(TC // P):
                    psv = ps_a.tile([P, HPC * DH], f32, tag="a", name="psv")
                    for dc in range(NDC):
                        nc.tensor.matmul(
                            psv,
                            xt[:, dc, tb * P:(tb + 1) * P],
                            w_sb[:, dc, 4 * P:6 * P],
                            start=(dc == 0),
                            stop=(dc == NDC - 1),
                        )
                    nc.scalar.copy(v_sb[:, tci * (TC // P) + tb, :], psv)

            if b == 0:
                load_late_consts()

            # ---------------- attention per head ----------------
            u_sb = []
            for h in range(HPC):
                us = up.tile([P, S], f32r, tag=f"u{h}", name=f"u{h}")
                for c in range(NQC):
                    psu = ps_u.tile([P, QC], f32, tag="u", name="psu")
                    psd = ps_d.tile([P, QC], f32, tag="d", name="psd")
                    last_kb = KPQ * c + KPQ - 1
                    for kb in range(last_kb + 1):
                        diag = kb >= KPQ * c
                        off = (kb - KPQ * c) * P if diag else 0
                        psp = ps_a.tile([P, QC], f32, tag="a", name="psp")
                        nc.tensor.matmul(
                            psp[:, off:],
                            k_sb[h][:, kb * P:(kb + 1) * P],
                            q_sb[h][:, c * QC + off:(c + 1) * QC],
                            start=True,
                            stop=True,
                        )
                        p_t = pp.tile([P, QC], bf16, tag="p", name="p_t")
                        nc.scalar.activation(
                            p_t[:, off:], psp[:, off:], Act.Exp, scale=SCALE
                        )
                        if diag:
                            nc.gpsimd.tensor_mul(
                                p_t[:, off:off + P],
                                p_t[:, off:off + P],
                                tril_sb,
                            )
                        # denominator: ones^T @ p -> [1, qc], accumulated
                        nc.tensor.matmul(
                            psd[:, off:],
                            onec_sb,
                            p_t[:, off:],
                            start=(kb == 0),
                            stop=(kb == last_kb),
                        )
                        # attn @ V in [dh, q] layout, accumulated
                        nc.tensor.matmul(
                            psu[:, off:],
                            v_sb[:, kb, h * DH:(h + 1) * DH],
                            p_t[:, off:],
                            start=(kb == 0),
                            stop=(kb == last_kb),
                        )
                    # ---- chunk postlude: 1/denominator (already broadcast
                    # across partitions by the ones matmul), evict u
                    rb_sb = rbp.tile([P, QC], f32, tag="rb", name="rb_sb")
                    nc.vector.reciprocal_approx_fast(out=rb_sb, in_=psd)
                    nc.vector.scalar_tensor_tensor(
                        us[:, c * QC:(c + 1) * QC],
                        psu,
                        1.0,
                        rb_sb,
                        op0=Alu.mult,
                        op1=Alu.mult,
                    )
                u_sb.append(us)

            # ---------------- out projection ----------------
            u0 = u_sb[0]
            u1 = u_sb[1]
            for tb in range(NTB):
                f_t = fin.tile([P, D], bf16, tag="f", name="f_t")
                for cc in range(D // QC):
                    pso = ps_a.tile([P, QC], f32, tag="a", name="pso")
                    nc.tensor.matmul(
                        pso,
                        u0[:, tb * P:(tb + 1) * P],
                        wo_sb[:, 0, cc * QC:(cc + 1) * QC],
                        start=True,
                        stop=False,
                    )
                    nc.tensor.matmul(
                        pso,
                        u1[:, tb * P:(tb + 1) * P],
                        wo_sb[:, 1, cc * QC:(cc + 1) * QC],
                        start=False,
                        stop=True,
                    )
                    dst = f_t[:, cc * QC:(cc + 1) * QC]
                    if cc % 2 == 0:
                        nc.scalar.copy(dst, pso)
                    else:
                        nc.vector.tensor_copy(out=dst, in_=pso)
                nc.sync.dma_start(out_p[b, tb * P:(tb + 1) * P, :], f_t)


_CACHE = {}


def _build():
    if "nc" in _CACHE:
        return _CACHE["nc"]
    nc = bacc.Bacc("TRN2", target_bir_lowering=False, debug=False)
    xty = nc.dram_tensor("xty", [B, D, S], f32, kind="ExternalInput").ap()
    wqkv = nc.dram_tensor("wqkv", [D, 6 * P], f32, kind="ExternalInput").ap()
    bqk = nc.dram_tensor("bqk", [P, 4], f32, kind="ExternalInput").ap()
    wout = nc.dram_tensor("wout", [HPC * DH, D], f32, kind="ExternalInput").ap()
    trilm = nc.dram_tensor("trilm", [P, P], bf16, kind="ExternalInput").ap()
    onesc = nc.dram_tensor("onesc", [P, P], bf16, kind="ExternalInput").ap()
    out_p = nc.dram_tensor("out_p", [B, S, D], bf16, kind="ExternalOutput").ap()
    with tile.TileContext(nc) as tctx:
        _emit(nc, tctx, (xty, wqkv, bqk, wout, trilm, onesc, out_p))
    nc.compile()
    _CACHE["nc"] = nc
    return nc


def _in_maps(x, W_qkv, b_qkv, W_out, b_out):
    import ml_dtypes
    trilm = np.where(
        np.arange(P)[None, :] >= np.arange(P)[:, None], 1.0, 0.0
    ).astype(ml_dtypes.bfloat16)
    onesc = np.ones((P, P), dtype=ml_dtypes.bfloat16)
    xty = np.ascontiguousarray(x.transpose(0, 2, 1))
    maps = []
    for core in range(NCORES):
        h0 = core * HPC
        cols = []
        for off in (0, D, 2 * D):  # q, k, v column groups of W_qkv
            for h in range(h0, h0 + HPC):
                cols.append((off + h * DH, off + (h + 1) * DH))
        wqkv_c = np.concatenate(
            [W_qkv[:, a:b_] for a, b_ in cols], axis=1
        ).astype(np.float32)
        bqk_c = np.stack(
            [b_qkv[a:b_] for a, b_ in cols[:4]], axis=1
        ).astype(np.float32)  # [128, 4]
        wout_c = W_out[h0 * DH:(h0 + HPC) * DH, :].astype(np.float32)
        maps.append({
            "xty": xty,
            "wqkv": np.ascontiguousarray(wqkv_c),
            "bqk": np.ascontiguousarray(bqk_c),
            "wout": np.ascontiguousarray(wout_c),
            "trilm": trilm,
            "onesc": onesc,
        })
    return maps


def kernel(x, W_qkv, b_qkv, W_out, b_out, _trace=False, _trace_kwargs=None):
    x = np.asarray(x, dtype=np.float32)
    W_qkv = np.asarray(W_qkv, dtype=np.float32)
    b_qkv = np.asarray(b_qkv, dtype=np.float32)
    W_out = np.asarray(W_out, dtype=np.float32)
    b_out = np.asarray(b_out, dtype=np.float32)

    nc = _build()
    maps = _in_maps(x, W_qkv, b_qkv, W_out, b_out)
    res = run_bass_kernel_spmd(
        nc, maps, core_ids=list(range(NCORES)), trace=_trace,
        **(_trace_kwargs or {}),
    )
    out = res.results[0]["out_p"].astype(np.float32)
    for c in range(1, NCORES):
        out = out + res.results[c]["out_p"].astype(np.float32)
    # all biases folded on the host: b_out + (v-bias @ W_out); exact because
    # softmax rows sum to 1, so the v-bias passes through attention unchanged
    bias_total = (b_out + b_qkv[2 * D:] @ W_out).astype(np.float32)
    out = out + bias_total[None, None, :]
    if _trace:
        _CACHE["last_results"] = res
    return out.astype(np.float32)
